# revision 5
# baseline (speedup 1.0000x reference)
"""Trainium2 Bass kernel for the BASE_LIF_SNN problem.

Computation (reference):
    for t in range(T):
        ff = x_t @ W1.T
        v  = leak_v * v * (1 - s) + ff
        s  = (v - thresh > 0)
        u  = leak_i * u + s @ W2.T
    outputs: spikes [T,B,N], states_snn [T,2,B,N] (stack of v, s), decoded [T,B,N]

Strategy (per core, data-parallel over B: 8 cores x B_local=8):
  - Normalize by thresh: w = v / thresh, fold 1/thresh into W1 (host).
    Spike test becomes (w > 1.0), so the reset fuses into a single
    scalar_tensor_tensor op with an immediate scalar.
  - FF = X @ W1n.T computed in bulk on PE in f32r with a hi/lo split
    (3 products) for ~fp32 accuracy at 1 cycle/row.
  - The only serial part is a 3-op/step DVE scan over T:
        w_t   = rho_{t-1} + ffn_t          (tensor_tensor add)
        h_t   = w_t * leakB                (tensor_tensor mult)
        rho_t = (w_t <= 1) * h_t           (scalar_tensor_tensor)
  - Spikes s = (w > 1) batched on GPSIMD (f32r output feeds Y matmul).
  - Y = S @ W2.T on PE (f32r), then u-scan via one tensor_tensor_scan per
    n-chunk; a leak-mask (zero at each segment start) restarts the
    recurrence at batch segment boundaries so one instruction scans all 8.
  - w and u stream out in lane-major layout (contiguous DMA); host
    reassembles [T,B,N], rescales v = w * thresh, and recomputes
    s = (w > 1) bit-identically.

Lane layout per core: lane (b, n) -> partition p = n % 128, m = n // 128.
  Stage tiles are [128, 2048] per chunk of 64 steps:
    ff/w/s stage col = m*512 + t*8 + b
    y/u stage col    = m*512 + b*64 + t
Time axis padded to 1024 (16 uniform chunks); host slices back to T=1000.
"""
import sys

if '/opt/trn_rl_repo' not in sys.path:
    sys.path.insert(0, '/opt/trn_rl_repo')

import numpy as np

B, T, N = 64, 1000, 512
NCORES = 8
BL = B // NCORES          # 8 batch rows per core
TC = 64                   # time steps per chunk
TPAD = 1024
NCHUNK = TPAD // TC       # 16 uniform chunks
M = N // 128              # 4 n-chunks


def _rnd11(x):
    """Round fp32 to the float32r grid (11 explicit mantissa bits)."""
    b = x.view(np.uint32).astype(np.uint64)
    out = ((b + np.uint64(1 << 11)) & np.uint64(0xFFFFF000)).astype(np.uint32)
    return out.view(np.float32)


_CACHED = {}


def _build():
    import concourse.bacc as bacc
    import concourse.mybir as mybir
    from concourse.tile import TileContext

    F32 = mybir.dt.float32
    F32R = mybir.dt.float32r
    ALU = mybir.AluOpType

    nc = bacc.Bacc()

    # x pre-packed on host: [M, NCHUNK, 128, 512] with [ci, c, p, b*64+t]
    xhi = nc.declare_dram_parameter("xhi", [M, NCHUNK, 128, 512], F32R, isOutput=False)
    xlo = nc.declare_dram_parameter("xlo", [M, NCHUNK, 128, 512], F32R, isOutput=False)
    w1hi = nc.declare_dram_parameter("w1hi", [N, N], F32R, isOutput=False)   # [n_in, n_out]
    w1lo = nc.declare_dram_parameter("w1lo", [N, N], F32R, isOutput=False)
    w2t = nc.declare_dram_parameter("w2t", [N, N], F32R, isOutput=False)     # [n_in, n_out]
    leakb = nc.declare_dram_parameter("leakb", [128, 32], F32, isOutput=False)
    rho0 = nc.declare_dram_parameter("rho0", [128, 32], F32, isOutput=False)
    ld0 = nc.declare_dram_parameter("ld0", [128, M * 512], F32, isOutput=False)
    uinit = nc.declare_dram_parameter("uinit", [128, M * 512], F32, isOutput=False)

    wout = nc.declare_dram_parameter("wout", [NCHUNK, 128, 2048], F32, isOutput=True)
    uout = nc.declare_dram_parameter("uout", [NCHUNK, 128, 2048], F32, isOutput=True)

    with TileContext(nc) as tc:
        with (
            tc.tile_pool(name="const", bufs=1) as cp,
            tc.tile_pool(name="xin", bufs=2) as xp,
            tc.tile_pool(name="stage", bufs=2) as sp,
            tc.tile_pool(name="ustage", bufs=2) as up,
            tc.tile_pool(name="psf", bufs=2, space="PSUM") as psf,
            tc.tile_pool(name="psy", bufs=2, space="PSUM") as psy,
        ):
            # ---- constants ----
            w1hi_t = [cp.tile([128, N], F32R, name=f"w1hi{ci}") for ci in range(M)]
            w1lo_t = [cp.tile([128, N], F32R, name=f"w1lo{ci}") for ci in range(M)]
            w2t_t = [cp.tile([128, N], F32R, name=f"w2t{ci}") for ci in range(M)]
            for ci in range(M):
                nc.sync.dma_start(out=w1hi_t[ci][:], in_=w1hi[ci * 128:(ci + 1) * 128, :])
                nc.sync.dma_start(out=w1lo_t[ci][:], in_=w1lo[ci * 128:(ci + 1) * 128, :])
                nc.sync.dma_start(out=w2t_t[ci][:], in_=w2t[ci * 128:(ci + 1) * 128, :])
            leakb_t = cp.tile([128, 32], F32, name="leakb_t")
            nc.sync.dma_start(out=leakb_t[:], in_=leakb[:])
            ld0_t = cp.tile([128, M * 512], F32, name="ld0_t")
            nc.sync.dma_start(out=ld0_t[:], in_=ld0[:])
            uinit_t = cp.tile([128, M * 512], F32, name="uinit_t")
            nc.sync.dma_start(out=uinit_t[:], in_=uinit[:])
            rho = cp.tile([128, 32], F32, name="rho")
            nc.sync.dma_start(out=rho[:], in_=rho0[:])
            h = cp.tile([128, 32], F32, name="h")

            leakb_v = leakb_t[:].rearrange("p (m b) -> p m b", m=M)
            rho_v = rho[:].rearrange("p (m b) -> p m b", m=M)
            h_v = h[:].rearrange("p (m b) -> p m b", m=M)

            u_prev = None
            for c in range(NCHUNK):
                # ---- DMA in: x chunk tiles [128, 512] per ci (contiguous)
                xhi_t = [xp.tile([128, 512], F32R, name=f"xhi{ci}", tag=f"xhi{ci}")
                         for ci in range(M)]
                xlo_t = [xp.tile([128, 512], F32R, name=f"xlo{ci}", tag=f"xlo{ci}")
                         for ci in range(M)]
                for ci in range(M):
                    nc.sync.dma_start(out=xhi_t[ci][:], in_=xhi[ci, c])
                    nc.sync.dma_start(out=xlo_t[ci][:], in_=xlo[ci, c])

                # ---- FF matmuls: psum[m] [128, b*64+t] = sum_ci pieces
                ffn_stage = sp.tile([128, 2048], F32, name="ffn_stage", tag="ffn")
                for m in range(M):
                    pt = psf.tile([128, 512], F32, name=f"psff{m}", tag="psff")
                    for ci in range(M):
                        nc.tensor.matmul(pt[:], w1hi_t[ci][:, m * 128:(m + 1) * 128],
                                         xhi_t[ci][:], start=(ci == 0), stop=False)
                    for ci in range(M):
                        nc.tensor.matmul(pt[:], w1lo_t[ci][:, m * 128:(m + 1) * 128],
                                         xhi_t[ci][:], start=False, stop=False)
                    for ci in range(M):
                        nc.tensor.matmul(pt[:], w1hi_t[ci][:, m * 128:(m + 1) * 128],
                                         xlo_t[ci][:], start=False, stop=(ci == M - 1))
                    # evac psum (col b*64+t) -> ffn_stage cols m*512 + t*8 + b
                    dst = ffn_stage[:, m * 512:(m + 1) * 512]
                    dst = dst.rearrange("p (t b) -> p b t", b=BL)
                    nc.scalar.activation(
                        dst, pt[:].rearrange("p (b t) -> p b t", b=BL),
                        mybir.ActivationFunctionType.Copy)

                # ---- DVE scan over this chunk
                w_stage = sp.tile([128, 2048], F32, name="w_stage", tag="wst")
                for t in range(TC):
                    ff_v = ffn_stage[:].rearrange("p (m t b) -> p m t b", m=M, b=BL)[:, :, t, :]
                    w_v = w_stage[:].rearrange("p (m t b) -> p m t b", m=M, b=BL)[:, :, t, :]
                    nc.vector.tensor_tensor(out=w_v, in0=rho_v, in1=ff_v, op=ALU.add)
                    nc.vector.tensor_tensor(out=h_v, in0=w_v, in1=leakb_v, op=ALU.mult)
                    nc.vector.scalar_tensor_tensor(
                        out=rho_v, in0=w_v, scalar=1.0, in1=h_v,
                        op0=ALU.is_le, op1=ALU.mult)

                # ---- spikes (f32r, feeds Y matmul)
                s_stage = sp.tile([128, 2048], F32R, name="s_stage", tag="sst")
                nc.vector.tensor_scalar(out=s_stage[:], in0=w_stage[:],
                                        scalar1=1.0, scalar2=None, op0=ALU.is_gt)

                # ---- Y matmuls: psum[m] [128, b*64+t] = sum_ci W2[:,m].T @ s
                y_stage = up.tile([128, 2048], F32, name="y_stage", tag="yst")
                for m in range(M):
                    pt = psy.tile([128, 512], F32, name=f"psy{m}", tag="psy")
                    for ci in range(M):
                        rhs = s_stage[:, ci * 512:(ci + 1) * 512]
                        rhs = rhs.rearrange("p (t b) -> p b t", b=BL)
                        nc.tensor.matmul(pt[:].rearrange("p (b t) -> p b t", b=BL),
                                         w2t_t[ci][:, m * 128:(m + 1) * 128],
                                         rhs, start=(ci == 0), stop=(ci == M - 1))
                    nc.scalar.activation(
                        y_stage[:, m * 512:(m + 1) * 512], pt[:],
                        mybir.ActivationFunctionType.Copy)

                # ---- u-scan: chain-fix first column of each b segment, then TTS per m
                u_stage = up.tile([128, 2048], F32, name="u_stage", tag="ust")
                prev = uinit_t if u_prev is None else u_prev
                for m in range(M):
                    # y[:, m*512 + b*64 + 0] += leak_i[m] * u_prev[:, m*512 + b*64 + 63]
                    y0 = y_stage[:].rearrange("p (m b t) -> p m b t", m=M, t=TC)[:, m, :, 0]
                    up_last = prev[:].rearrange("p (m b t) -> p m b t", m=M, t=TC)[:, m, :, TC - 1]
                    nc.vector.scalar_tensor_tensor(
                        out=y0, in0=up_last, scalar=ld0_t[:, m * 512 + 1: m * 512 + 2],
                        in1=y0, op0=ALU.mult, op1=ALU.add)
                    # TTS over the whole m-region (b-major, t-fast; ld0 zeroes leak at each t=0)
                    nc.vector.tensor_tensor_scan(
                        out=u_stage[:, m * 512:(m + 1) * 512],
                        data0=ld0_t[:, m * 512:(m + 1) * 512],
                        data1=y_stage[:, m * 512:(m + 1) * 512],
                        initial=0.0, op0=ALU.mult, op1=ALU.add)
                u_prev = u_stage

                # ---- DMA out
                for m in range(M):
                    nc.sync.dma_start(out=wout[c, :, m * 512:(m + 1) * 512],
                                      in_=w_stage[:, m * 512:(m + 1) * 512])
                    nc.sync.dma_start(out=uout[c, :, m * 512:(m + 1) * 512],
                                      in_=u_stage[:, m * 512:(m + 1) * 512])
    nc.compile()
    return nc


def _get_nc():
    if "nc" not in _CACHED:
        _CACHED["nc"] = _build()
    return _CACHED["nc"]


def _pack_x(x_core):
    """[BL, T, N] fp32 -> [M, NCHUNK, 128, 512] with [ci, c, p, b*64+t]."""
    xp = np.zeros((BL, TPAD, N), np.float32)
    xp[:, :T] = x_core
    # [b, c, t, m, p] -> [m, c, p, b, t]
    v = xp.reshape(BL, NCHUNK, TC, M, 128).transpose(3, 1, 4, 0, 2)
    return np.ascontiguousarray(v.reshape(M, NCHUNK, 128, 512))


def kernel(input_batch, state_snn, state_LI, W1, W2, leak_v, leak_i, thresh):
    from concourse.bass_utils import run_bass_kernel_spmd

    input_batch = np.asarray(input_batch, dtype=np.float32)
    state_snn = np.asarray(state_snn, dtype=np.float32)
    state_LI = np.asarray(state_LI, dtype=np.float32)
    W1 = np.asarray(W1, dtype=np.float32)
    W2 = np.asarray(W2, dtype=np.float32)
    leak_v = np.asarray(leak_v, dtype=np.float32)
    leak_i = np.asarray(leak_i, dtype=np.float32)
    thresh = np.asarray(thresh, dtype=np.float32)

    # ---- host prep ----
    w1n_t = np.ascontiguousarray((W1 / thresh[:, None]).T)   # [n_in, n_out]
    w1hi = _rnd11(w1n_t)
    w1lo = np.ascontiguousarray(w1n_t - w1hi)
    w2t = np.ascontiguousarray(W2.T)                          # [n_in, n_out]

    # lane tiles: lane (b, n): p = n % 128, m = n // 128
    n_of = (np.arange(M)[:, None] * 128 + np.arange(128)[None, :])  # [m, p]
    leakb = np.zeros((128, 32), np.float32)                   # col = m*8+b
    for m in range(M):
        leakb[:, m * 8:(m + 1) * 8] = leak_v[n_of[m]][:, None]
    ld0 = np.zeros((128, M * 512), np.float32)                # col = m*512 + b*64 + t
    for m in range(M):
        blk = np.repeat(leak_i[n_of[m]][:, None], 512, axis=1).reshape(128, BL, TC)
        blk[:, :, 0] = 0.0
        ld0[:, m * 512:(m + 1) * 512] = blk.reshape(128, 512)

    v0, s0 = state_snn[0], state_snn[1]                       # [B, N]
    rho_full = leak_v[None, :] * v0 * (1.0 - s0) / thresh[None, :]   # [B, N]

    in_maps = []
    for core in range(NCORES):
        bsl = slice(core * BL, (core + 1) * BL)
        x_core = np.ascontiguousarray(input_batch[bsl])       # [BL, T, N]
        xhi = _rnd11(x_core)
        xlo = np.ascontiguousarray(x_core - xhi)
        rho0 = np.zeros((128, 32), np.float32)
        uinit = np.zeros((128, M * 512), np.float32)
        for m in range(M):
            rho0[:, m * 8:(m + 1) * 8] = rho_full[bsl][:, n_of[m]].T
            ui = state_LI[bsl][:, n_of[m]].T                  # [128, BL]
            blk = np.zeros((128, BL, TC), np.float32)
            blk[:, :, TC - 1] = ui
            uinit[:, m * 512:(m + 1) * 512] = blk.reshape(128, 512)
        in_maps.append({
            "xhi": _pack_x(xhi), "xlo": _pack_x(xlo),
            "w1hi": w1hi, "w1lo": w1lo, "w2t": w2t,
            "leakb": leakb, "rho0": rho0, "ld0": ld0, "uinit": uinit,
        })

    nc = _get_nc()
    res = run_bass_kernel_spmd(nc, in_maps, list(range(NCORES)))
    _CACHED["last_results"] = res

    # ---- host reassembly ----
    spikes = np.empty((T, B, N), np.float32)
    v_full = np.empty((T, B, N), np.float32)
    decoded = np.empty((T, B, N), np.float32)
    for core in range(NCORES):
        bsl = slice(core * BL, (core + 1) * BL)
        wsc = res.results[core]["wout"]                       # [NCHUNK, 128, 2048]
        usc = res.results[core]["uout"]
        # w: col = m*512 + t*8 + b -> [c, p, m, t, b] -> [c, t, b, m, p]
        wr = wsc.reshape(NCHUNK, 128, M, TC, BL)
        wr = wr.transpose(0, 3, 4, 2, 1).reshape(TPAD, BL, N)[:T]
        v_full[:, bsl] = wr
        # u: col = m*512 + b*64 + t -> [c, p, m, b, t] -> [c, t, b, m, p]
        ur = usc.reshape(NCHUNK, 128, M, BL, TC)
        ur = ur.transpose(0, 4, 3, 2, 1).reshape(TPAD, BL, N)[:T]
        decoded[:, bsl] = ur
    spikes[:] = (v_full > 1.0).astype(np.float32)
    v_full *= thresh[None, None, :]
    states_snn = np.stack([v_full, spikes], axis=1)           # [T, 2, B, N]
    return spikes, states_snn, decoded
